# Initial kernel scaffold
#
"""GNN message-passing (2x GraphConv + attention gate + mean-pool classifier)
on 8 trn2 NeuronCores via Bass/Tile.

Strategy (dst-partitioned):
  - nodes sharded 12500/core; each core owns aggregation for its dst nodes.
  - per layer: each core computes h_s = (h @ W) * norm_out for its shard
    (node-major, bf16), AllGather -> full gather table in DRAM.
  - edges (bucketed by dst core on host) are processed as 128-edge tiles
    grouped by (src-chunk, dst-block): dma_gather fetches message rows
    (256B bf16) + one-hot rows (256B bf16 from a small identity table);
    a PE matmul per tile (lhsT=one-hot [128e x 128slot], rhs=messages
    [128e x 64f]) accumulates into PSUM per segment; segments add into an
    SBUF accumulator m_sb.
  - post-pass: h_next = relu(m * norm_in + b) (* norm_out for next layer).
  - epilogue: sigmoid gate, per-graph one-hot pooling matmul, AllReduce,
    classifier matmul.

Host-side preprocessing is index/graph-structure only (degrees, norms,
edge bucketing, one-hot tables) plus exact regroupings (x * norm_out,
transpose for layout).
"""

import math
import os

import numpy as np

_KCACHE = {}


# ---------------------------------------------------------------- config ---
class Cfg:
    def __init__(
        self,
        N=100000,
        E=1600000,
        G=64,
        D=128,
        H=64,
        NCLS=10,
        C=8,
        CHUNK=32768,
        CALL=4096,
    ):
        self.N, self.E, self.G, self.D, self.H, self.NCLS, self.C = N, E, G, D, H, NCLS, C
        assert N % C == 0
        self.NP = N // C  # nodes per core
        self.NT = (self.NP + 127) // 128  # node tiles per core
        self.NPAD = self.NT * 128
        self.CHUNK = CHUNK
        self.NCHUNK = (N + CHUNK - 1) // CHUNK
        self.NB = (self.NP + 127) // 128  # dst blocks per core (== NT)
        self.CALL = CALL  # max edges per dma_gather call


# ------------------------------------------------------------ host preproc ---
def preprocess(x, edge_index, graph_id, cfg):
    """Returns per-core input maps' index/const tensors + T_common grid."""
    import ml_dtypes

    c = cfg
    src = np.asarray(edge_index[0], np.int64)
    dst = np.asarray(edge_index[1], np.int64)
    gid = np.asarray(graph_id, np.int64)

    deg_out = np.bincount(src, minlength=c.N).astype(np.float64)
    deg_in = np.bincount(dst, minlength=c.N).astype(np.float64)
    norm_out = (1.0 / np.sqrt(np.clip(deg_out, 1.0, None))).astype(np.float32)
    norm_in = (1.0 / np.sqrt(np.clip(deg_in, 1.0, None))).astype(np.float32)

    cnt = np.bincount(gid, minlength=c.G).astype(np.float32)
    inv_cnt = (1.0 / np.clip(cnt, 1.0, None)).astype(np.float32)

    core = dst // c.NP
    dst_loc = dst - core * c.NP
    blk = dst_loc >> 7
    dst_rel = dst_loc & 127
    chunk = src // c.CHUNK
    src_rel = (src % c.CHUNK).astype(np.int64)

    # counts per (core, chunk, blk)
    key = (core * c.NCHUNK + chunk) * c.NB + blk
    cnts = np.bincount(key, minlength=c.C * c.NCHUNK * c.NB).reshape(
        c.C, c.NCHUNK, c.NB
    )
    # tiles needed per (chunk, blk): max over cores
    T = np.maximum((cnts + 127) // 128, 0).max(axis=0)  # [NCHUNK, NB]

    # order edges per core by (chunk, blk)
    order = np.lexsort((blk, chunk, core))
    src_rel_s = src_rel[order]
    dst_rel_s = dst_rel[order]
    core_s = core[order]
    chunk_s = chunk[order]
    blk_s = blk[order]

    # stream layout per core: for chunk k: for blk b: T[k][b]*128 slots
    seg_cap = T * 128  # [NCHUNK, NB]
    seg_off = np.zeros((c.NCHUNK, c.NB), np.int64)
    off = 0
    for k in range(c.NCHUNK):
        for b in range(c.NB):
            seg_off[k, b] = off
            off += seg_cap[k, b]
    CK = int(off)  # total stream positions per core

    gq = np.zeros((c.C, CK), np.int16)  # gather idx (chunk-relative); pad -> 0
    sq = np.full((c.C, CK), 128, np.int16)  # one-hot idx; pad -> 128 (zero row)

    # fill: position within each (core, chunk, blk) run
    # edges are sorted by (core, chunk, blk) so runs are contiguous
    run_key = (core_s * c.NCHUNK + chunk_s) * c.NB + blk_s
    # position within run
    first = np.r_[True, run_key[1:] != run_key[:-1]]
    run_start = np.maximum.accumulate(np.where(first, np.arange(len(run_key)), 0))
    pos_in_run = np.arange(len(run_key)) - run_start
    stream_pos = seg_off[chunk_s, blk_s] + pos_in_run
    gq[core_s, stream_pos] = src_rel_s.astype(np.int16)
    sq[core_s, stream_pos] = dst_rel_s.astype(np.int16)

    # per-chunk call split
    calls = []  # list of (chunk, offset, n)
    for k in range(c.NCHUNK):
        k_off = int(seg_off[k, 0])
        k_len = int(seg_cap[k].sum())
        o = 0
        while o < k_len:
            n = min(c.CALL, k_len - o)
            calls.append((k, k_off + o, n))
            o += n

    # wrap idx streams into the [128, CK/16] call-local layout
    def wrap_calls(q):
        w = np.zeros((16, CK // 16), np.int16)
        for (_, o, n) in calls:
            w[:, o // 16 : (o + n) // 16] = q[o : o + n].reshape(-1, 16).T
        return np.tile(w, (8, 1))

    in_maps = []
    for cc in range(c.C):
        lo, hi = cc * c.NP, (cc + 1) * c.NP
        xs = np.asarray(x[lo:hi], np.float32) * norm_out[lo:hi, None]
        xT = np.zeros((c.D, c.NPAD), np.float32)
        xT[:, : c.NP] = xs.T
        ni = np.zeros((128, c.NT), np.float32)
        no = np.zeros((128, c.NT), np.float32)
        ni.T.reshape(-1)[: c.NP] = norm_in[lo:hi]
        no.T.reshape(-1)[: c.NP] = norm_out[lo:hi]
        goh = np.zeros((c.NPAD, c.G), np.float32)
        goh[np.arange(c.NP), gid[lo:hi]] = 1.0
        in_maps.append(
            {
                "xT": xT,
                "ni_t": np.ascontiguousarray(ni),
                "no_t": np.ascontiguousarray(no),
                "gonehot": goh,
                "gq": wrap_calls(gq[cc]),
                "sq": wrap_calls(sq[cc]),
            }
        )

    meta = dict(
        T=T,
        calls=calls,
        CK=CK,
        norm_out=norm_out,
        norm_in=norm_in,
        inv_cnt=inv_cnt,
    )
    return in_maps, meta


# ------------------------------------------------------------- bass kernel ---
def build_kernel(cfg, T, calls, CK):
    import concourse.bacc as bacc
    import concourse.mybir as mybir
    import concourse.tile as tile

    c = cfg
    f32 = mybir.dt.float32
    bf16 = mybir.dt.bfloat16
    i16 = mybir.dt.int16
    AF = mybir.ActivationFunctionType

    nc = bacc.Bacc("TRN2", target_bir_lowering=False, debug=False, num_devices=c.C)
    nc.dynamic_dma_scratch_size = 2**17

    # ---- parameters
    xT_in = nc.declare_dram_parameter("xT", [c.D, c.NPAD], f32, isOutput=False)
    W1_in = nc.declare_dram_parameter("W1", [c.D, c.H], f32, isOutput=False)
    W2_in = nc.declare_dram_parameter("W2", [c.H, c.H], f32, isOutput=False)
    b1b_in = nc.declare_dram_parameter("b1b", [128, c.H], f32, isOutput=False)
    b2b_in = nc.declare_dram_parameter("b2b", [128, c.H], f32, isOutput=False)
    awb_in = nc.declare_dram_parameter("awb", [128, c.H], f32, isOutput=False)
    clsW_in = nc.declare_dram_parameter("clsW", [c.H, c.NCLS], f32, isOutput=False)
    clsbb_in = nc.declare_dram_parameter("clsbb", [c.G, c.NCLS], f32, isOutput=False)
    icb_in = nc.declare_dram_parameter("icb", [c.H, c.G], f32, isOutput=False)
    attnb_in = nc.declare_dram_parameter("attnb", [128, 1], f32, isOutput=False)
    ident_in = nc.declare_dram_parameter("ident", [128, 128], f32, isOutput=False)
    id2_in = nc.declare_dram_parameter("id2", [129, 128], bf16, isOutput=False)
    ni_in = nc.declare_dram_parameter("ni_t", [128, c.NT], f32, isOutput=False)
    no_in = nc.declare_dram_parameter("no_t", [128, c.NT], f32, isOutput=False)
    goh_in = nc.declare_dram_parameter("gonehot", [c.NPAD, c.G], f32, isOutput=False)
    gq_in = nc.declare_dram_parameter("gq", [128, CK // 16], i16, isOutput=False)
    sq_in = nc.declare_dram_parameter("sq", [128, CK // 16], i16, isOutput=False)

    out_out = nc.declare_dram_parameter("out", [c.G, c.NCLS], f32, isOutput=True)
    hw_out = nc.declare_dram_parameter("hw", [c.NPAD, 1], f32, isOutput=True)

    # ---- internal DRAM
    bounce = [nc.dram_tensor(f"tbl_bounce{l}", [c.NP, 128], bf16) for l in (0, 1)]
    tbl = [
        nc.dram_tensor(f"table{l}", [c.N, 128], bf16, addr_space="Shared")
        for l in (0, 1)
    ]
    ar_in = nc.dram_tensor("ar_in", [c.H, c.G], f32)
    ar_out = nc.dram_tensor("ar_out", [c.H, c.G], f32, addr_space="Shared")

    CALLS_BY_CHUNK = {}
    for (k, o, n) in calls:
        CALLS_BY_CHUNK.setdefault(k, []).append((o, n))

    nseg_tiles = [[int(T[k][b]) for b in range(c.NB)] for k in range(c.NCHUNK)]

    with tile.TileContext(nc) as tc:
        with (
            tc.tile_pool(name="const", bufs=1) as cpool,
            tc.tile_pool(name="big", bufs=1) as bigpool,
            tc.tile_pool(name="stream", bufs=3) as spool,
            tc.tile_pool(name="idx", bufs=3) as ipool,
            tc.tile_pool(name="small", bufs=4) as mpool,
            tc.tile_pool(name="ps", bufs=4, space="PSUM") as psp,
            tc.tile_pool(name="ps_acc", bufs=2, space="PSUM") as psacc,
        ):
            # ---------- load constants
            W1 = cpool.tile([c.D, c.H], f32, tag="W1")
            W2 = cpool.tile([c.H, c.H], f32, tag="W2")
            b1b = cpool.tile([128, c.H], f32, tag="b1b")
            b2b = cpool.tile([128, c.H], f32, tag="b2b")
            awb = cpool.tile([128, c.H], f32, tag="awb")
            clsW = cpool.tile([c.H, c.NCLS], f32, tag="clsW")
            clsbb = cpool.tile([c.G, c.NCLS], f32, tag="clsbb")
            icb = cpool.tile([c.H, c.G], f32, tag="icb")
            attnb = cpool.tile([128, 1], f32, tag="attnb")
            ident = cpool.tile([128, 128], f32, tag="ident")
            nit = cpool.tile([128, c.NT], f32, tag="nit")
            not_ = cpool.tile([128, c.NT], f32, tag="not")
            for t, s in (
                (W1, W1_in), (W2, W2_in), (b1b, b1b_in), (b2b, b2b_in),
                (awb, awb_in), (clsW, clsW_in), (clsbb, clsbb_in), (icb, icb_in),
                (attnb, attnb_in), (ident, ident_in), (nit, ni_in), (not_, no_in),
            ):
                nc.sync.dma_start(out=t[:], in_=s[:])

            # ---------- resident SBUF
            m_sb = bigpool.tile([128, c.NB * c.H], f32, tag="m_sb")
            h1s = bigpool.tile([128, c.NT * c.H], f32, tag="h1s")
            h2 = bigpool.tile([128, c.NT * c.H], f32, tag="h2")

            # ---------- helper: table build layer 0 (from xT via W1)
            def table_build(l):
                for t in range(c.NT):
                    lo = t * 128
                    hi = min(c.NP, lo + 128)
                    if l == 0:
                        lhsT = spool.tile([c.D, 128], f32, tag="xT_t")
                        nc.sync.dma_start(out=lhsT[:], in_=xT_in[:, lo : lo + 128])
                        W = W1
                        kdim = c.D
                    else:
                        # transpose h1s tile -> [H, 128] then lhsT
                        pt = psp.tile([c.H, 128], f32, tag="ps_tr")
                        nc.tensor.transpose(
                            pt[:], h1s[:, t * c.H : (t + 1) * c.H], ident[:]
                        )
                        lhsT = spool.tile([c.H, 128], f32, tag="h1sT_t")
                        nc.scalar.activation(lhsT[:], pt[:], AF.Copy)
                        W = W2
                        kdim = c.H
                    po = psp.tile([128, c.H], f32, tag="ps_tbl")
                    nc.tensor.matmul(
                        po[:], lhsT[:kdim, :], W[:], start=True, stop=True
                    )
                    ob = spool.tile([128, 128], bf16, tag="tblb")
                    nc.scalar.activation(ob[:, 0 : c.H], po[:], AF.Copy)
                    nc.sync.dma_start(
                        out=bounce[l][lo:hi, :], in_=ob[0 : hi - lo, :]
                    )

            # ---------- helper: gather+aggregate into m_sb
            def aggregate(l):
                nc.vector.memset(m_sb[:], 0.0)
                id2s = cpool.tile([129, 128], bf16, tag="id2")
                if l == 0:
                    nc.sync.dma_start(out=id2s[:], in_=id2_in[:])
                # segment cursor: walk calls and segments in lockstep
                seg_list = []  # flattened (k, b, ntiles)
                for k in range(c.NCHUNK):
                    for b in range(c.NB):
                        if nseg_tiles[k][b]:
                            seg_list.append((k, b, nseg_tiles[k][b]))
                seg_i = 0
                seg_left = seg_list[0][2] if seg_list else 0
                pacc = None
                for (k, o, n) in calls:
                    nt = n // 128
                    gt = spool.tile([128, c.CALL // 128, 128], bf16, tag="gstream")
                    st = spool.tile([128, c.CALL // 128, 128], bf16, tag="sstream")
                    gi = ipool.tile([128, c.CALL // 16], i16, tag="gidx")
                    si = ipool.tile([128, c.CALL // 16], i16, tag="sidx")
                    nc.sync.dma_start(
                        out=gi[:, 0 : n // 16], in_=gq_in[:, o // 16 : (o + n) // 16]
                    )
                    nc.sync.dma_start(
                        out=si[:, 0 : n // 16], in_=sq_in[:, o // 16 : (o + n) // 16]
                    )
                    rows0 = k * c.CHUNK
                    rows1 = min(c.N, rows0 + c.CHUNK)
                    nc.gpsimd.dma_gather(
                        gt[:, 0:nt, :],
                        tbl[l][rows0:rows1, :],
                        gi[:, 0 : n // 16],
                        n,
                        n,
                        128,
                        single_packet=False,
                    )
                    nc.gpsimd.dma_gather(
                        st[:, 0:nt, :],
                        id2s[:, :] if False else id2_in[:, :],
                        si[:, 0 : n // 16],
                        n,
                        n,
                        128,
                        single_packet=False,
                    )
                    for s in range(nt):
                        if seg_left == seg_list[seg_i][2]:
                            pacc = psacc.tile([128, c.H], f32, tag="ps_m")
                        start = seg_left == seg_list[seg_i][2]
                        seg_left -= 1
                        stop = seg_left == 0
                        nc.tensor.matmul(
                            pacc[:],
                            st[:, s, :],
                            gt[:, s, 0 : c.H],
                            start=start,
                            stop=stop,
                        )
                        if stop:
                            b = seg_list[seg_i][1]
                            nc.vector.tensor_add(
                                m_sb[:, b * c.H : (b + 1) * c.H],
                                pacc[:],
                                m_sb[:, b * c.H : (b + 1) * c.H],
                            )
                            seg_i += 1
                            seg_left = (
                                seg_list[seg_i][2] if seg_i < len(seg_list) else 0
                            )

            # ---------- helper: post-pass
            def postpass(l):
                bb = b1b if l == 0 else b2b
                dst = h1s if l == 0 else h2
                for t in range(c.NT):
                    sl = slice(t * c.H, (t + 1) * c.H)
                    t1 = mpool.tile([128, c.H], f32, tag="pp1")
                    nc.vector.tensor_scalar_mul(t1[:], m_sb[:, sl], nit[:, t : t + 1])
                    t2 = mpool.tile([128, c.H], f32, tag="pp2")
                    nc.vector.tensor_add(t2[:], t1[:], bb[:])
                    if l == 0:
                        nc.scalar.activation(
                            dst[:, sl], t2[:], AF.Relu, scale=not_[:, t : t + 1]
                        )
                    else:
                        nc.scalar.activation(dst[:, sl], t2[:], AF.Relu)

            # ================= layer 1 =================
            table_build(0)
            nc.gpsimd.collective_compute(
                "AllGather",
                mybir.AluOpType.bypass,
                replica_groups=[list(range(c.C))],
                ins=[bounce[0][:, :]],
                outs=[tbl[0][:, :]],
            )
            aggregate(0)
            postpass(0)

            # ================= layer 2 =================
            table_build(1)
            nc.gpsimd.collective_compute(
                "AllGather",
                mybir.AluOpType.bypass,
                replica_groups=[list(range(c.C))],
                ins=[bounce[1][:, :]],
                outs=[tbl[1][:, :]],
            )
            aggregate(1)
            postpass(1)

            # ================= attention + pooling =================
            ppool = psacc.tile([c.H, c.G], f32, tag="ps_pool")
            for t in range(c.NT):
                sl = slice(t * c.H, (t + 1) * c.H)
                tmp = mpool.tile([128, c.H], f32, tag="at1")
                nc.vector.tensor_mul(tmp[:], h2[:, sl], awb[:])
                red = mpool.tile([128, 1], f32, tag="at2")
                nc.vector.reduce_sum(red[:], tmp[:], axis=mybir.AxisListType.X)
                hwt = mpool.tile([128, 1], f32, tag="at3")
                nc.scalar.activation(hwt[:], red[:], AF.Sigmoid, bias=attnb[:])
                lo = t * 128
                nc.sync.dma_start(out=hw_out[lo : lo + 128, :], in_=hwt[:])
                hg = mpool.tile([128, c.H], f32, tag="at4")
                nc.vector.tensor_scalar_mul(hg[:], h2[:, sl], hwt[:])
                go = mpool.tile([128, c.G], f32, tag="at5")
                nc.sync.dma_start(out=go[:], in_=goh_in[lo : lo + 128, :])
                nc.tensor.matmul(
                    ppool[:], hg[:], go[:], start=(t == 0), stop=(t == c.NT - 1)
                )
            pooled = mpool.tile([c.H, c.G], f32, tag="pooled")
            nc.vector.tensor_copy(pooled[:], ppool[:])
            nc.sync.dma_start(out=ar_in[:, :], in_=pooled[:])
            nc.gpsimd.collective_compute(
                "AllReduce",
                mybir.AluOpType.add,
                replica_groups=[list(range(c.C))],
                ins=[ar_in[:, :]],
                outs=[ar_out[:, :]],
            )
            hgT = mpool.tile([c.H, c.G], f32, tag="hgT")
            nc.sync.dma_start(out=hgT[:], in_=ar_out[:, :])
            hgTs = mpool.tile([c.H, c.G], f32, tag="hgTs")
            nc.vector.tensor_mul(hgTs[:], hgT[:], icb[:])
            pout = psp.tile([c.G, c.NCLS], f32, tag="ps_out")
            nc.tensor.matmul(pout[:], hgTs[:], clsW[:], start=True, stop=True)
            ob = mpool.tile([c.G, c.NCLS], f32, tag="outb")
            nc.vector.tensor_add(ob[:], pout[:], clsbb[:])
            nc.sync.dma_start(out=out_out[:, :], in_=ob[:])

    nc.compile()
    return nc


# ----------------------------------------------------------------- driver ---
def _const_inputs(cfg, W1, b1, W2, b2, attn_W, attn_b, cls_W, cls_b, inv_cnt):
    import ml_dtypes

    c = cfg
    consts = {
        "W1": np.asarray(W1, np.float32),
        "W2": np.asarray(W2, np.float32),
        "b1b": np.tile(np.asarray(b1, np.float32)[None, :], (128, 1)),
        "b2b": np.tile(np.asarray(b2, np.float32)[None, :], (128, 1)),
        "awb": np.tile(np.asarray(attn_W, np.float32)[:, 0][None, :], (128, 1)),
        "clsW": np.asarray(cls_W, np.float32),
        "clsbb": np.tile(np.asarray(cls_b, np.float32)[None, :], (c.G, 1)),
        "icb": np.tile(inv_cnt[None, :], (c.H, 1)),
        "attnb": np.full((128, 1), np.float32(np.asarray(attn_b).reshape(-1)[0])),
        "ident": np.eye(128, dtype=np.float32),
        "id2": np.vstack(
            [np.eye(128, dtype=np.float32), np.zeros((1, 128), np.float32)]
        ).astype(ml_dtypes.bfloat16),
    }
    return consts


def run(x, edge_index, graph_id, W1, b1, W2, b2, attn_W, attn_b, cls_W, cls_b,
        cfg=None):
    from concourse.bass_utils import run_bass_kernel_spmd

    cfg = cfg or Cfg()
    in_maps, meta = preprocess(x, edge_index, graph_id, cfg)
    consts = _const_inputs(
        cfg, W1, b1, W2, b2, attn_W, attn_b, cls_W, cls_b, meta["inv_cnt"]
    )
    for m in in_maps:
        m.update(consts)

    ckey = (cfg.N, cfg.E, tuple(map(tuple, meta["T"])), len(meta["calls"]))
    if ckey not in _KCACHE:
        _KCACHE[ckey] = build_kernel(cfg, meta["T"], meta["calls"], meta["CK"])
    nc = _KCACHE[ckey]

    res = run_bass_kernel_spmd(
        nc, in_maps, core_ids=list(range(cfg.C)),
        trace=os.environ.get("KTRACE", "0") == "1",
    )
    out = res.results[0]["out"]
    hw = np.concatenate(
        [res.results[cc]["hw"][: cfg.NP] for cc in range(cfg.C)], axis=0
    )
    return out, hw, res


def kernel(**inputs):
    out, hw, _ = run(**inputs)
    return out, hw


# revision 13
# speedup vs baseline: 1.0385x; 1.0385x over previous
"""GNN message-passing (2x GraphConv + attention gate + mean-pool classifier)
on 8 trn2 NeuronCores via Bass/Tile.

Strategy (dst-partitioned):
  - nodes sharded 12500/core; each core owns aggregation for its dst nodes.
  - per layer: each core computes h_s = (h @ W) * norm_out for its shard
    (node-major, bf16), AllGather -> full gather table in DRAM.
  - edges (bucketed by dst core on host) are processed as 128-edge tiles
    grouped by (src-chunk, dst-block): dma_gather fetches message rows
    (256B bf16) + one-hot rows (256B bf16 from a small identity table);
    a PE matmul per tile (lhsT=one-hot [128e x 128slot], rhs=messages
    [128e x 64f]) accumulates into PSUM per segment; segments add into an
    SBUF accumulator m_sb.
  - post-pass: h_next = relu(m * norm_in + b) (* norm_out for next layer).
  - epilogue: sigmoid gate, per-graph one-hot pooling matmul, AllReduce,
    classifier matmul.

Host-side preprocessing is index/graph-structure only (degrees, norms,
edge bucketing, one-hot tables) plus exact regroupings (x * norm_out,
transpose for layout).
"""

import math
import os

import numpy as np

_KCACHE = {}


# ---------------------------------------------------------------- config ---
class Cfg:
    def __init__(
        self,
        N=100000,
        E=1600000,
        G=64,
        D=128,
        H=64,
        NCLS=10,
        C=8,
        CHUNK=32768,
        CALL=4096,
    ):
        self.N, self.E, self.G, self.D, self.H, self.NCLS, self.C = N, E, G, D, H, NCLS, C
        assert N % C == 0
        self.NP = N // C  # nodes per core
        self.NT = (self.NP + 127) // 128  # node tiles per core
        self.NPAD = self.NT * 128
        self.CHUNK = CHUNK
        self.NCHUNK = (N + CHUNK - 1) // CHUNK
        self.NB = (self.NP + 127) // 128  # dst blocks per core (== NT)
        self.CALL = CALL  # max edges per dma_gather call


# ------------------------------------------------------------ host preproc ---
def preprocess(x, edge_index, graph_id, cfg):
    """Returns per-core input maps' index/const tensors + T_common grid."""
    import ml_dtypes

    c = cfg
    src = np.asarray(edge_index[0], np.int64)
    dst = np.asarray(edge_index[1], np.int64)
    gid = np.asarray(graph_id, np.int64)

    deg_out = np.bincount(src, minlength=c.N).astype(np.float64)
    deg_in = np.bincount(dst, minlength=c.N).astype(np.float64)
    norm_out = (1.0 / np.sqrt(np.clip(deg_out, 1.0, None))).astype(np.float32)
    norm_in = (1.0 / np.sqrt(np.clip(deg_in, 1.0, None))).astype(np.float32)

    cnt = np.bincount(gid, minlength=c.G).astype(np.float32)
    inv_cnt = (1.0 / np.clip(cnt, 1.0, None)).astype(np.float32)

    core = dst // c.NP
    dst_loc = dst - core * c.NP
    blk = dst_loc >> 7
    dst_rel = dst_loc & 127
    chunk = src // c.CHUNK
    src_rel = (src % c.CHUNK).astype(np.int64)

    # counts per (core, chunk, blk)
    key = (core * c.NCHUNK + chunk) * c.NB + blk
    cnts = np.bincount(key, minlength=c.C * c.NCHUNK * c.NB).reshape(
        c.C, c.NCHUNK, c.NB
    )
    # tiles needed per (chunk, blk): max over cores
    T = np.maximum((cnts + 127) // 128, 0).max(axis=0)  # [NCHUNK, NB]

    # order edges per core by (chunk, blk)
    order = np.lexsort((blk, chunk, core))
    src_rel_s = src_rel[order]
    dst_rel_s = dst_rel[order]
    core_s = core[order]
    chunk_s = chunk[order]
    blk_s = blk[order]

    # stream layout per core: for chunk k: for blk b: T[k][b]*128 slots
    seg_cap = T * 128  # [NCHUNK, NB]
    seg_off = np.zeros((c.NCHUNK, c.NB), np.int64)
    off = 0
    for k in range(c.NCHUNK):
        for b in range(c.NB):
            seg_off[k, b] = off
            off += seg_cap[k, b]
    CK = int(off)  # total stream positions per core

    gq = np.zeros((c.C, CK), np.int16)  # gather idx (chunk-relative); pad -> 0
    sq = np.full((c.C, CK), 128, np.int16)  # one-hot idx; pad -> 128 (zero row)

    # fill: position within each (core, chunk, blk) run
    # edges are sorted by (core, chunk, blk) so runs are contiguous
    run_key = (core_s * c.NCHUNK + chunk_s) * c.NB + blk_s
    # position within run
    first = np.r_[True, run_key[1:] != run_key[:-1]]
    run_start = np.maximum.accumulate(np.where(first, np.arange(len(run_key)), 0))
    pos_in_run = np.arange(len(run_key)) - run_start
    stream_pos = seg_off[chunk_s, blk_s] + pos_in_run
    gq[core_s, stream_pos] = src_rel_s.astype(np.int16)
    sq[core_s, stream_pos] = dst_rel_s.astype(np.int16)

    # per-chunk call split
    calls = []  # list of (chunk, offset, n)
    for k in range(c.NCHUNK):
        k_off = int(seg_off[k, 0])
        k_len = int(seg_cap[k].sum())
        o = 0
        while o < k_len:
            n = min(c.CALL, k_len - o)
            calls.append((k, k_off + o, n))
            o += n

    # wrap idx streams into the [128, CK/16] call-local layout
    def wrap_calls(q):
        w = np.zeros((16, CK // 16), np.int16)
        for (_, o, n) in calls:
            w[:, o // 16 : (o + n) // 16] = q[o : o + n].reshape(-1, 16).T
        return np.tile(w, (8, 1))

    in_maps = []
    for cc in range(c.C):
        lo, hi = cc * c.NP, (cc + 1) * c.NP
        xs = np.asarray(x[lo:hi], np.float32) * norm_out[lo:hi, None]
        xT = np.zeros((c.D, c.NPAD), np.float32)
        xT[:, : c.NP] = xs.T
        nif = np.zeros(c.NPAD, np.float32)
        nof = np.zeros(c.NPAD, np.float32)
        nif[: c.NP] = norm_in[lo:hi]
        nof[: c.NP] = norm_out[lo:hi]
        ni = np.ascontiguousarray(nif.reshape(c.NT, 128).T)
        no = np.ascontiguousarray(nof.reshape(c.NT, 128).T)
        goh = np.zeros((c.NPAD, c.G), np.float32)
        goh[np.arange(c.NP), gid[lo:hi]] = 1.0
        in_maps.append(
            {
                "xT": xT,
                "ni_t": ni,
                "no_t": no,
                "gonehot": goh,
                "gq": wrap_calls(gq[cc]),
                "sq": wrap_calls(sq[cc]),
            }
        )

    meta = dict(
        T=T,
        calls=calls,
        CK=CK,
        norm_out=norm_out,
        norm_in=norm_in,
        inv_cnt=inv_cnt,
    )
    return in_maps, meta


# ------------------------------------------------------------- bass kernel ---
def build_kernel(cfg, T, calls, CK):
    import concourse.bacc as bacc
    import concourse.mybir as mybir
    import concourse.tile as tile

    c = cfg
    f32 = mybir.dt.float32
    bf16 = mybir.dt.bfloat16
    i16 = mybir.dt.int16
    AF = mybir.ActivationFunctionType

    nc = bacc.Bacc("TRN2", target_bir_lowering=False, debug=False, num_devices=c.C)
    nc.dynamic_dma_scratch_size = 2**17

    # ---- parameters
    xT_in = nc.declare_dram_parameter("xT", [c.D, c.NPAD], f32, isOutput=False)
    W1_in = nc.declare_dram_parameter("W1", [c.D, c.H], f32, isOutput=False)
    W2_in = nc.declare_dram_parameter("W2", [c.H, c.H], f32, isOutput=False)
    b1b_in = nc.declare_dram_parameter("b1b", [128, c.H], f32, isOutput=False)
    b2b_in = nc.declare_dram_parameter("b2b", [128, c.H], f32, isOutput=False)
    awb_in = nc.declare_dram_parameter("awb", [128, c.H], f32, isOutput=False)
    clsW_in = nc.declare_dram_parameter("clsW", [c.H, c.NCLS], f32, isOutput=False)
    clsbb_in = nc.declare_dram_parameter("clsbb", [c.G, c.NCLS], f32, isOutput=False)
    icb_in = nc.declare_dram_parameter("icb", [c.H, c.G], f32, isOutput=False)
    attnb_in = nc.declare_dram_parameter("attnb", [128, 1], f32, isOutput=False)
    ident_in = nc.declare_dram_parameter("ident", [128, 128], f32, isOutput=False)
    id2_in = nc.declare_dram_parameter("id2", [129, 128], bf16, isOutput=False)
    ni_in = nc.declare_dram_parameter("ni_t", [128, c.NT], f32, isOutput=False)
    no_in = nc.declare_dram_parameter("no_t", [128, c.NT], f32, isOutput=False)
    goh_in = nc.declare_dram_parameter("gonehot", [c.NPAD, c.G], f32, isOutput=False)
    gq_in = nc.declare_dram_parameter("gq", [128, CK // 16], i16, isOutput=False)
    sq_in = nc.declare_dram_parameter("sq", [128, CK // 16], i16, isOutput=False)

    out_out = nc.declare_dram_parameter("out", [c.G, c.NCLS], f32, isOutput=True)
    hw_out = nc.declare_dram_parameter("hw", [c.NPAD, 1], f32, isOutput=True)
    dbg = os.environ.get("KDEBUG", "0") == "1"
    if dbg:
        mdump_out = nc.declare_dram_parameter(
            "m_dump", [c.NB * 128, c.H], f32, isOutput=True
        )
        hdump_out = nc.declare_dram_parameter(
            "h_dump", [c.NT * 128, c.H], f32, isOutput=True
        )

    # ---- internal DRAM
    bounce = [nc.dram_tensor(f"tbl_bounce{l}", [c.NP, 128], bf16) for l in (0, 1)]
    tbl = [
        nc.dram_tensor(f"table{l}", [c.N, 128], bf16, addr_space="Shared")
        for l in (0, 1)
    ]
    ar_in = nc.dram_tensor("ar_in", [c.H, c.G], f32)
    ar_out = nc.dram_tensor("ar_out", [c.H, c.G], f32, addr_space="Shared")

    CALLS_BY_CHUNK = {}
    for (k, o, n) in calls:
        CALLS_BY_CHUNK.setdefault(k, []).append((o, n))

    nseg_tiles = [[int(T[k][b]) for b in range(c.NB)] for k in range(c.NCHUNK)]

    with tile.TileContext(nc) as tc:
        with (
            tc.tile_pool(name="const", bufs=1) as cpool,
            tc.tile_pool(name="big", bufs=1) as bigpool,
            tc.tile_pool(name="stream", bufs=3) as spool,
            tc.tile_pool(name="idx", bufs=3) as ipool,
            tc.tile_pool(name="small", bufs=4) as mpool,
            tc.tile_pool(name="ps", bufs=2, space="PSUM") as psp,
            tc.tile_pool(name="ps_acc", bufs=2, space="PSUM") as psacc,
        ):
            # ---------- load constants
            W1 = cpool.tile([c.D, c.H], f32, tag="W1")
            W2 = cpool.tile([c.H, c.H], f32, tag="W2")
            b1b = cpool.tile([128, c.H], f32, tag="b1b")
            b2b = cpool.tile([128, c.H], f32, tag="b2b")
            awb = cpool.tile([128, c.H], f32, tag="awb")
            clsW = cpool.tile([c.H, c.NCLS], f32, tag="clsW")
            clsbb = cpool.tile([c.G, c.NCLS], f32, tag="clsbb")
            icb = cpool.tile([c.H, c.G], f32, tag="icb")
            attnb = cpool.tile([128, 1], f32, tag="attnb")
            ident = cpool.tile([128, 128], f32, tag="ident")
            nit = cpool.tile([128, c.NT], f32, tag="nit")
            not_ = cpool.tile([128, c.NT], f32, tag="not")
            for t, s in (
                (W1, W1_in), (W2, W2_in), (b1b, b1b_in), (b2b, b2b_in),
                (awb, awb_in), (clsW, clsW_in), (clsbb, clsbb_in), (icb, icb_in),
                (attnb, attnb_in), (ident, ident_in), (nit, ni_in), (not_, no_in),
            ):
                nc.sync.dma_start(out=t[:], in_=s[:])

            # ---------- resident SBUF
            m_sb = bigpool.tile([128, c.NB * c.H], f32, tag="m_sb")
            h1s = bigpool.tile([128, c.NT * c.H], f32, tag="h1s")
            h2 = bigpool.tile([128, c.NT * c.H], f32, tag="h2")

            # ---------- helper: table build layer 0 (from xT via W1)
            def table_build(l):
                for t in range(c.NT):
                    lo = t * 128
                    hi = min(c.NP, lo + 128)
                    if l == 0:
                        lhsT = spool.tile([c.D, 128], f32, tag="xT_t")
                        nc.sync.dma_start(out=lhsT[:], in_=xT_in[:, lo : lo + 128])
                        W = W1
                        kdim = c.D
                    else:
                        # transpose h1s tile -> [H, 128] then lhsT
                        pt = psp.tile([c.H, 128], f32, tag="ps_tr")
                        nc.tensor.transpose(
                            pt[:], h1s[:, t * c.H : (t + 1) * c.H], ident[:]
                        )
                        lhsT = spool.tile([c.H, 128], f32, tag="h1sT_t")
                        nc.scalar.activation(lhsT[:], pt[:], AF.Copy)
                        W = W2
                        kdim = c.H
                    po = psp.tile([128, c.H], f32, tag="ps_tbl")
                    nc.tensor.matmul(
                        po[:], lhsT[:kdim, :], W[:], start=True, stop=True
                    )
                    ob = spool.tile([128, 128], bf16, tag="tblb")
                    nc.scalar.activation(ob[:, 0 : c.H], po[:], AF.Copy)
                    nc.scalar.activation(ob[:, c.H : 2 * c.H], po[:], AF.Copy)
                    nc.sync.dma_start(
                        out=bounce[l][lo:hi, :], in_=ob[0 : hi - lo, :]
                    )

            # ---------- helper: gather+aggregate into m_sb
            def aggregate(l):
                nc.vector.memset(m_sb[:], 0.0)
                # segment cursor: walk calls and segments in lockstep
                seg_list = []  # flattened (k, b, ntiles)
                for k in range(c.NCHUNK):
                    for b in range(c.NB):
                        if nseg_tiles[k][b]:
                            seg_list.append((k, b, nseg_tiles[k][b]))
                seg_i = 0
                seg_left = seg_list[0][2] if seg_list else 0
                pacc = None
                for (k, o, n) in calls:
                    nt = n // 128
                    gt = spool.tile([128, c.CALL // 128, 128], bf16, tag="gstream")
                    st = spool.tile([128, c.CALL // 128, 128], bf16, tag="sstream")
                    gi = ipool.tile([128, c.CALL // 16], i16, tag="gidx")
                    si = ipool.tile([128, c.CALL // 16], i16, tag="sidx")
                    nc.sync.dma_start(
                        out=gi[:, 0 : n // 16], in_=gq_in[:, o // 16 : (o + n) // 16]
                    )
                    nc.sync.dma_start(
                        out=si[:, 0 : n // 16], in_=sq_in[:, o // 16 : (o + n) // 16]
                    )
                    rows0 = k * c.CHUNK
                    rows1 = min(c.N, rows0 + c.CHUNK)
                    nc.gpsimd.dma_gather(
                        gt[:, 0:nt, :],
                        tbl[l][rows0:rows1, :],
                        gi[:, 0 : n // 16],
                        n,
                        n,
                        128,
                        single_packet=False,
                    )
                    nc.gpsimd.dma_gather(
                        st[:, 0:nt, :],
                        id2_in[:, :],
                        si[:, 0 : n // 16],
                        n,
                        n,
                        128,
                        single_packet=False,
                    )
                    for s in range(nt):
                        if seg_left == seg_list[seg_i][2]:
                            pacc = psacc.tile([128, c.H], f32, tag="ps_m")
                        start = seg_left == seg_list[seg_i][2]
                        seg_left -= 1
                        stop = seg_left == 0
                        nc.tensor.matmul(
                            pacc[:],
                            st[:, s, :],
                            gt[:, s, 0 : c.H],
                            start=start,
                            stop=stop,
                        )
                        if stop:
                            b = seg_list[seg_i][1]
                            nc.vector.tensor_add(
                                m_sb[:, b * c.H : (b + 1) * c.H],
                                pacc[:],
                                m_sb[:, b * c.H : (b + 1) * c.H],
                            )
                            seg_i += 1
                            seg_left = (
                                seg_list[seg_i][2] if seg_i < len(seg_list) else 0
                            )

            # ---------- helper: post-pass
            def postpass(l):
                bb = b1b if l == 0 else b2b
                dst = h1s if l == 0 else h2
                for t in range(c.NT):
                    sl = slice(t * c.H, (t + 1) * c.H)
                    t1 = mpool.tile([128, c.H], f32, tag="pp1")
                    nc.vector.tensor_scalar_mul(t1[:], m_sb[:, sl], nit[:, t : t + 1])
                    t2 = mpool.tile([128, c.H], f32, tag="pp2")
                    nc.vector.tensor_add(t2[:], t1[:], bb[:])
                    if l == 0:
                        nc.scalar.activation(
                            dst[:, sl], t2[:], AF.Relu, scale=not_[:, t : t + 1]
                        )
                    else:
                        nc.scalar.activation(dst[:, sl], t2[:], AF.Relu)

            # ================= layer 1 =================
            table_build(0)
            nc.gpsimd.collective_compute(
                "AllGather",
                mybir.AluOpType.bypass,
                replica_groups=[list(range(c.C))],
                ins=[bounce[0][:, :]],
                outs=[tbl[0][:, :]],
            )
            aggregate(0)
            if dbg:
                for b in range(c.NB):
                    mdt = mpool.tile([128, c.H], f32, tag="mdump")
                    nc.vector.tensor_copy(mdt[:], m_sb[:, b * c.H : (b + 1) * c.H])
                    nc.sync.dma_start(
                        out=mdump_out[b * 128 : (b + 1) * 128, :], in_=mdt[:]
                    )
            postpass(0)
            if dbg:
                for t in range(c.NT):
                    hdt = mpool.tile([128, c.H], f32, tag="hdump")
                    nc.vector.tensor_copy(hdt[:], h1s[:, t * c.H : (t + 1) * c.H])
                    nc.sync.dma_start(
                        out=hdump_out[t * 128 : (t + 1) * 128, :], in_=hdt[:]
                    )

            # ================= layer 2 =================
            table_build(1)
            nc.gpsimd.collective_compute(
                "AllGather",
                mybir.AluOpType.bypass,
                replica_groups=[list(range(c.C))],
                ins=[bounce[1][:, :]],
                outs=[tbl[1][:, :]],
            )
            aggregate(1)
            postpass(1)

            # ================= attention + pooling =================
            ppool = psacc.tile([c.H, c.G], f32, tag="ps_m")
            for t in range(c.NT):
                sl = slice(t * c.H, (t + 1) * c.H)
                tmp = mpool.tile([128, c.H], f32, tag="at1")
                nc.vector.tensor_mul(tmp[:], h2[:, sl], awb[:])
                red = mpool.tile([128, 1], f32, tag="at2")
                nc.vector.reduce_sum(red[:], tmp[:], axis=mybir.AxisListType.X)
                hwt = mpool.tile([128, 1], f32, tag="at3")
                nc.scalar.activation(hwt[:], red[:], AF.Sigmoid, bias=attnb[:])
                lo = t * 128
                nc.sync.dma_start(out=hw_out[lo : lo + 128, :], in_=hwt[:])
                hg = mpool.tile([128, c.H], f32, tag="at4")
                nc.vector.tensor_scalar_mul(hg[:], h2[:, sl], hwt[:])
                go = mpool.tile([128, c.G], f32, tag="at5")
                nc.sync.dma_start(out=go[:], in_=goh_in[lo : lo + 128, :])
                nc.tensor.matmul(
                    ppool[:], hg[:], go[:], start=(t == 0), stop=(t == c.NT - 1)
                )
            pooled = mpool.tile([c.H, c.G], f32, tag="pooled")
            nc.vector.tensor_copy(pooled[:], ppool[:])
            nc.sync.dma_start(out=ar_in[:, :], in_=pooled[:])
            nc.gpsimd.collective_compute(
                "AllReduce",
                mybir.AluOpType.add,
                replica_groups=[list(range(c.C))],
                ins=[ar_in[:, :]],
                outs=[ar_out[:, :]],
            )
            hgT = mpool.tile([c.H, c.G], f32, tag="hgT")
            nc.sync.dma_start(out=hgT[:], in_=ar_out[:, :])
            hgTs = mpool.tile([c.H, c.G], f32, tag="hgTs")
            nc.vector.tensor_mul(hgTs[:], hgT[:], icb[:])
            pout = psp.tile([c.G, c.NCLS], f32, tag="ps_tbl")
            nc.tensor.matmul(pout[:], hgTs[:], clsW[:], start=True, stop=True)
            ob = mpool.tile([c.G, c.NCLS], f32, tag="outb")
            nc.vector.tensor_add(ob[:], pout[:], clsbb[:])
            nc.sync.dma_start(out=out_out[:, :], in_=ob[:])

    nc.compile()
    return nc


# ----------------------------------------------------------------- driver ---
def _const_inputs(cfg, W1, b1, W2, b2, attn_W, attn_b, cls_W, cls_b, inv_cnt):
    import ml_dtypes

    c = cfg
    consts = {
        "W1": np.asarray(W1, np.float32),
        "W2": np.asarray(W2, np.float32),
        "b1b": np.tile(np.asarray(b1, np.float32)[None, :], (128, 1)),
        "b2b": np.tile(np.asarray(b2, np.float32)[None, :], (128, 1)),
        "awb": np.tile(np.asarray(attn_W, np.float32)[:, 0][None, :], (128, 1)),
        "clsW": np.asarray(cls_W, np.float32),
        "clsbb": np.tile(np.asarray(cls_b, np.float32)[None, :], (c.G, 1)),
        "icb": np.tile(inv_cnt[None, :], (c.H, 1)),
        "attnb": np.full((128, 1), np.float32(np.asarray(attn_b).reshape(-1)[0])),
        "ident": np.eye(128, dtype=np.float32),
        "id2": np.vstack(
            [np.eye(128, dtype=np.float32), np.zeros((1, 128), np.float32)]
        ).astype(ml_dtypes.bfloat16),
    }
    return consts


def run(x, edge_index, graph_id, W1, b1, W2, b2, attn_W, attn_b, cls_W, cls_b,
        cfg=None):
    from concourse.bass_utils import run_bass_kernel_spmd

    cfg = cfg or Cfg()
    in_maps, meta = preprocess(x, edge_index, graph_id, cfg)
    consts = _const_inputs(
        cfg, W1, b1, W2, b2, attn_W, attn_b, cls_W, cls_b, meta["inv_cnt"]
    )
    for m in in_maps:
        m.update(consts)

    ckey = (cfg.N, cfg.E, tuple(map(tuple, meta["T"])), len(meta["calls"]))
    if ckey not in _KCACHE:
        _KCACHE[ckey] = build_kernel(cfg, meta["T"], meta["calls"], meta["CK"])
    nc = _KCACHE[ckey]

    res = run_bass_kernel_spmd(
        nc, in_maps, core_ids=list(range(cfg.C)),
        trace=os.environ.get("KTRACE", "0") == "1",
    )
    out = res.results[0]["out"]
    hw = np.concatenate(
        [res.results[cc]["hw"][: cfg.NP] for cc in range(cfg.C)], axis=0
    )
    return out, hw, res


def kernel(**inputs):
    out, hw, _ = run(**inputs)
    return out, hw


# revision 14
# speedup vs baseline: 1.4325x; 1.3794x over previous
"""GNN message-passing (2x GraphConv + attention gate + mean-pool classifier)
on 8 trn2 NeuronCores via Bass/Tile.

Strategy (dst-partitioned):
  - nodes sharded 12500/core; each core owns aggregation for its dst nodes.
  - per layer: each core computes h_s = (h @ W) * norm_out for its shard
    (node-major, bf16), AllGather -> full gather table in DRAM.
  - edges (bucketed by dst core on host) are processed as 128-edge tiles
    grouped by (src-chunk, dst-block): dma_gather fetches message rows
    (256B bf16) + one-hot rows (256B bf16 from a small identity table);
    a PE matmul per tile (lhsT=one-hot [128e x 128slot], rhs=messages
    [128e x 64f]) accumulates into PSUM per segment; segments add into an
    SBUF accumulator m_sb.
  - post-pass: h_next = relu(m * norm_in + b) (* norm_out for next layer).
  - epilogue: sigmoid gate, per-graph one-hot pooling matmul, AllReduce,
    classifier matmul.

Host-side preprocessing is index/graph-structure only (degrees, norms,
edge bucketing, one-hot tables) plus exact regroupings (x * norm_out,
transpose for layout).
"""

import math
import os

import numpy as np

_KCACHE = {}


# ---------------------------------------------------------------- config ---
class Cfg:
    def __init__(
        self,
        N=100000,
        E=1600000,
        G=64,
        D=128,
        H=64,
        NCLS=10,
        C=8,
        CHUNK=32768,
        CALL=4096,
    ):
        self.N, self.E, self.G, self.D, self.H, self.NCLS, self.C = N, E, G, D, H, NCLS, C
        assert N % C == 0
        self.NP = N // C  # nodes per core
        self.NT = (self.NP + 127) // 128  # node tiles per core
        self.NPAD = self.NT * 128
        self.CHUNK = CHUNK
        self.NCHUNK = (N + CHUNK - 1) // CHUNK
        self.NB = (self.NP + 127) // 128  # dst blocks per core (== NT)
        self.CALL = CALL  # max edges per dma_gather call


# ------------------------------------------------------------ host preproc ---
def preprocess(x, edge_index, graph_id, cfg):
    """Returns per-core input maps' index/const tensors + T_common grid."""
    import ml_dtypes

    c = cfg
    src = np.asarray(edge_index[0], np.int64)
    dst = np.asarray(edge_index[1], np.int64)
    gid = np.asarray(graph_id, np.int64)

    deg_out = np.bincount(src, minlength=c.N).astype(np.float64)
    deg_in = np.bincount(dst, minlength=c.N).astype(np.float64)
    norm_out = (1.0 / np.sqrt(np.clip(deg_out, 1.0, None))).astype(np.float32)
    norm_in = (1.0 / np.sqrt(np.clip(deg_in, 1.0, None))).astype(np.float32)

    cnt = np.bincount(gid, minlength=c.G).astype(np.float32)
    inv_cnt = (1.0 / np.clip(cnt, 1.0, None)).astype(np.float32)

    core = dst // c.NP
    dst_loc = dst - core * c.NP
    blk = dst_loc >> 7
    dst_rel = dst_loc & 127
    chunk = src // c.CHUNK
    src_rel = (src % c.CHUNK).astype(np.int64)

    # counts per (core, chunk, blk)
    key = (core * c.NCHUNK + chunk) * c.NB + blk
    cnts = np.bincount(key, minlength=c.C * c.NCHUNK * c.NB).reshape(
        c.C, c.NCHUNK, c.NB
    )
    # tiles needed per (chunk, blk): max over cores
    T = np.maximum((cnts + 127) // 128, 0).max(axis=0)  # [NCHUNK, NB]

    # order edges per core by (chunk, blk)
    order = np.lexsort((blk, chunk, core))
    src_rel_s = src_rel[order]
    dst_rel_s = dst_rel[order]
    core_s = core[order]
    chunk_s = chunk[order]
    blk_s = blk[order]

    # stream layout per core: for chunk k: for blk b: T[k][b]*128 slots
    seg_cap = T * 128  # [NCHUNK, NB]
    seg_off = np.zeros((c.NCHUNK, c.NB), np.int64)
    off = 0
    for k in range(c.NCHUNK):
        for b in range(c.NB):
            seg_off[k, b] = off
            off += seg_cap[k, b]
    CK = int(off)  # total stream positions per core

    gq = np.zeros((c.C, CK), np.int16)  # gather idx (chunk-relative); pad -> 0
    sq = np.full((c.C, CK), 128, np.int16)  # one-hot idx; pad -> 128 (zero row)

    # fill: position within each (core, chunk, blk) run
    # edges are sorted by (core, chunk, blk) so runs are contiguous
    run_key = (core_s * c.NCHUNK + chunk_s) * c.NB + blk_s
    # position within run
    first = np.r_[True, run_key[1:] != run_key[:-1]]
    run_start = np.maximum.accumulate(np.where(first, np.arange(len(run_key)), 0))
    pos_in_run = np.arange(len(run_key)) - run_start
    stream_pos = seg_off[chunk_s, blk_s] + pos_in_run
    gq[core_s, stream_pos] = src_rel_s.astype(np.int16)
    sq[core_s, stream_pos] = dst_rel_s.astype(np.int16)

    # per-chunk call split
    calls = []  # list of (chunk, offset, n)
    for k in range(c.NCHUNK):
        k_off = int(seg_off[k, 0])
        k_len = int(seg_cap[k].sum())
        o = 0
        while o < k_len:
            n = min(c.CALL, k_len - o)
            calls.append((k, k_off + o, n))
            o += n

    # wrap idx streams into the [128, CK/16] call-local layout
    def wrap_calls(q):
        w = np.zeros((16, CK // 16), np.int16)
        for (_, o, n) in calls:
            w[:, o // 16 : (o + n) // 16] = q[o : o + n].reshape(-1, 16).T
        return np.tile(w, (8, 1))

    in_maps = []
    for cc in range(c.C):
        lo, hi = cc * c.NP, (cc + 1) * c.NP
        xs = np.asarray(x[lo:hi], np.float32) * norm_out[lo:hi, None]
        xT = np.zeros((c.D, c.NPAD), np.float32)
        xT[:, : c.NP] = xs.T
        nif = np.zeros(c.NPAD, np.float32)
        nof = np.zeros(c.NPAD, np.float32)
        nif[: c.NP] = norm_in[lo:hi]
        nof[: c.NP] = norm_out[lo:hi]
        ni = np.ascontiguousarray(nif.reshape(c.NT, 128).T)
        no = np.ascontiguousarray(nof.reshape(c.NT, 128).T)
        goh = np.zeros((c.NPAD, c.G), np.float32)
        goh[np.arange(c.NP), gid[lo:hi]] = 1.0
        in_maps.append(
            {
                "xT": xT,
                "ni_t": ni,
                "no_t": no,
                "gonehot": goh,
                "gq": wrap_calls(gq[cc]),
                "sq": wrap_calls(sq[cc]),
            }
        )

    meta = dict(
        T=T,
        calls=calls,
        CK=CK,
        norm_out=norm_out,
        norm_in=norm_in,
        inv_cnt=inv_cnt,
    )
    return in_maps, meta


# ------------------------------------------------------------- bass kernel ---
def build_kernel(cfg, T, calls, CK):
    import concourse.bacc as bacc
    import concourse.mybir as mybir
    import concourse.tile as tile

    c = cfg
    f32 = mybir.dt.float32
    bf16 = mybir.dt.bfloat16
    i16 = mybir.dt.int16
    AF = mybir.ActivationFunctionType

    nc = bacc.Bacc("TRN2", target_bir_lowering=False, debug=False, num_devices=c.C)
    nc.dynamic_dma_scratch_size = 2**17

    # ---- parameters
    xT_in = nc.declare_dram_parameter("xT", [c.D, c.NPAD], f32, isOutput=False)
    W1_in = nc.declare_dram_parameter("W1", [c.D, c.H], f32, isOutput=False)
    W2_in = nc.declare_dram_parameter("W2", [c.H, c.H], f32, isOutput=False)
    b1b_in = nc.declare_dram_parameter("b1b", [128, c.H], f32, isOutput=False)
    b2b_in = nc.declare_dram_parameter("b2b", [128, c.H], f32, isOutput=False)
    awb_in = nc.declare_dram_parameter("awb", [128, c.H], f32, isOutput=False)
    clsW_in = nc.declare_dram_parameter("clsW", [c.H, c.NCLS], f32, isOutput=False)
    clsbb_in = nc.declare_dram_parameter("clsbb", [c.G, c.NCLS], f32, isOutput=False)
    icb_in = nc.declare_dram_parameter("icb", [c.H, c.G], f32, isOutput=False)
    attnb_in = nc.declare_dram_parameter("attnb", [128, 1], f32, isOutput=False)
    ident_in = nc.declare_dram_parameter("ident", [128, 128], f32, isOutput=False)
    id2_in = nc.declare_dram_parameter("id2", [129, 128], bf16, isOutput=False)
    ni_in = nc.declare_dram_parameter("ni_t", [128, c.NT], f32, isOutput=False)
    no_in = nc.declare_dram_parameter("no_t", [128, c.NT], f32, isOutput=False)
    goh_in = nc.declare_dram_parameter("gonehot", [c.NPAD, c.G], f32, isOutput=False)
    gq_in = nc.declare_dram_parameter("gq", [128, CK // 16], i16, isOutput=False)
    sq_in = nc.declare_dram_parameter("sq", [128, CK // 16], i16, isOutput=False)

    out_out = nc.declare_dram_parameter("out", [c.G, c.NCLS], f32, isOutput=True)
    hw_out = nc.declare_dram_parameter("hw", [c.NPAD, 1], f32, isOutput=True)
    dbg = os.environ.get("KDEBUG", "0") == "1"
    if dbg:
        mdump_out = nc.declare_dram_parameter(
            "m_dump", [c.NB * 128, c.H], f32, isOutput=True
        )
        hdump_out = nc.declare_dram_parameter(
            "h_dump", [c.NT * 128, c.H], f32, isOutput=True
        )

    # ---- internal DRAM
    bounce = [nc.dram_tensor(f"tbl_bounce{l}", [c.NP, 128], bf16) for l in (0, 1)]
    tbl = [
        nc.dram_tensor(f"table{l}", [c.N, 128], bf16, addr_space="Shared")
        for l in (0, 1)
    ]
    ar_in = nc.dram_tensor("ar_in", [c.H, c.G], f32)
    ar_out = nc.dram_tensor("ar_out", [c.H, c.G], f32, addr_space="Shared")

    CALLS_BY_CHUNK = {}
    for (k, o, n) in calls:
        CALLS_BY_CHUNK.setdefault(k, []).append((o, n))

    nseg_tiles = [[int(T[k][b]) for b in range(c.NB)] for k in range(c.NCHUNK)]

    with tile.TileContext(nc) as tc:
        with (
            tc.tile_pool(name="const", bufs=1) as cpool,
            tc.tile_pool(name="big", bufs=1) as bigpool,
            tc.tile_pool(name="stream", bufs=3) as spool,
            tc.tile_pool(name="idx", bufs=3) as ipool,
            tc.tile_pool(name="small", bufs=4) as mpool,
            tc.tile_pool(name="ps", bufs=2, space="PSUM") as psp,
            tc.tile_pool(name="ps_acc", bufs=2, space="PSUM") as psacc,
        ):
            # ---------- load constants
            W1 = cpool.tile([c.D, c.H], f32, tag="W1")
            W2 = cpool.tile([c.H, c.H], f32, tag="W2")
            b1b = cpool.tile([128, c.H], f32, tag="b1b")
            b2b = cpool.tile([128, c.H], f32, tag="b2b")
            awb = cpool.tile([128, c.H], f32, tag="awb")
            clsW = cpool.tile([c.H, c.NCLS], f32, tag="clsW")
            clsbb = cpool.tile([c.G, c.NCLS], f32, tag="clsbb")
            icb = cpool.tile([c.H, c.G], f32, tag="icb")
            attnb = cpool.tile([128, 1], f32, tag="attnb")
            ident = cpool.tile([128, 128], f32, tag="ident")
            nit = cpool.tile([128, c.NT], f32, tag="nit")
            not_ = cpool.tile([128, c.NT], f32, tag="not")
            for t, s in (
                (W1, W1_in), (W2, W2_in), (b1b, b1b_in), (b2b, b2b_in),
                (awb, awb_in), (clsW, clsW_in), (clsbb, clsbb_in), (icb, icb_in),
                (attnb, attnb_in), (ident, ident_in), (nit, ni_in), (not_, no_in),
            ):
                nc.sync.dma_start(out=t[:], in_=s[:])

            # ---------- resident SBUF
            m_sb = bigpool.tile([128, c.NB * c.H], f32, tag="m_sb")
            h1s = bigpool.tile([128, c.NT * c.H], f32, tag="h1s")
            h2 = bigpool.tile([128, c.NT * c.H], f32, tag="h2")

            # ---------- helper: table build layer 0 (from xT via W1)
            def table_build(l):
                for t in range(c.NT):
                    lo = t * 128
                    hi = min(c.NP, lo + 128)
                    if l == 0:
                        lhsT = spool.tile([c.D, 128], f32, tag="xT_t")
                        nc.sync.dma_start(out=lhsT[:], in_=xT_in[:, lo : lo + 128])
                        W = W1
                        kdim = c.D
                    else:
                        # transpose h1s tile -> [H, 128] then lhsT
                        pt = psp.tile([c.H, 128], f32, tag="ps_tr")
                        nc.tensor.transpose(
                            pt[:], h1s[:, t * c.H : (t + 1) * c.H], ident[:]
                        )
                        lhsT = spool.tile([c.H, 128], f32, tag="h1sT_t")
                        nc.scalar.activation(lhsT[:], pt[:], AF.Copy)
                        W = W2
                        kdim = c.H
                    po = psp.tile([128, c.H], f32, tag="ps_tbl")
                    nc.tensor.matmul(
                        po[:], lhsT[:kdim, :], W[:], start=True, stop=True
                    )
                    ob = spool.tile([128, 128], bf16, tag="tblb")
                    nc.scalar.activation(ob[:, 0 : c.H], po[:], AF.Copy)
                    nc.scalar.activation(ob[:, c.H : 2 * c.H], po[:], AF.Copy)
                    nc.sync.dma_start(
                        out=bounce[l][lo:hi, :], in_=ob[0 : hi - lo, :]
                    )

            # ---------- helper: gather+aggregate into m_sb
            def aggregate(l):
                KVAR = os.environ.get("KVAR", "full")
                nc.vector.memset(m_sb[:], 0.0)
                if KVAR == "noagg":
                    return
                # segment cursor: walk calls and segments in lockstep
                seg_list = []  # flattened (k, b, ntiles)
                for k in range(c.NCHUNK):
                    for b in range(c.NB):
                        if nseg_tiles[k][b]:
                            seg_list.append((k, b, nseg_tiles[k][b]))
                seg_i = 0
                seg_left = seg_list[0][2] if seg_list else 0
                pacc = None
                for (k, o, n) in calls:
                    nt = n // 128
                    gt = spool.tile([128, c.CALL // 128, 128], bf16, tag="gstream")
                    st = spool.tile([128, c.CALL // 128, 128], bf16, tag="sstream")
                    gi = ipool.tile([128, c.CALL // 16], i16, tag="gidx")
                    si = ipool.tile([128, c.CALL // 16], i16, tag="sidx")
                    nc.sync.dma_start(
                        out=gi[:, 0 : n // 16], in_=gq_in[:, o // 16 : (o + n) // 16]
                    )
                    nc.sync.dma_start(
                        out=si[:, 0 : n // 16], in_=sq_in[:, o // 16 : (o + n) // 16]
                    )
                    rows0 = k * c.CHUNK
                    rows1 = min(c.N, rows0 + c.CHUNK)
                    if KVAR != "nog":
                        nc.gpsimd.dma_gather(
                            gt[:, 0:nt, :],
                            tbl[l][rows0:rows1, :],
                            gi[:, 0 : n // 16],
                            n,
                            n,
                            128,
                            single_packet=False,
                        )
                    else:
                        nc.vector.memset(gt[:, 0:nt, :], 0.0)
                    if KVAR != "nos":
                        nc.gpsimd.dma_gather(
                            st[:, 0:nt, :],
                            id2_in[:, :],
                            si[:, 0 : n // 16],
                            n,
                            n,
                            128,
                            single_packet=False,
                        )
                    else:
                        nc.vector.memset(st[:, 0:nt, :], 0.0)
                    for s in range(nt):
                        if KVAR == "nomm":
                            continue
                        if seg_left == seg_list[seg_i][2]:
                            pacc = psacc.tile([128, c.H], f32, tag="ps_m")
                        start = seg_left == seg_list[seg_i][2]
                        seg_left -= 1
                        stop = seg_left == 0
                        nc.tensor.matmul(
                            pacc[:],
                            st[:, s, :],
                            gt[:, s, 0 : c.H],
                            start=start,
                            stop=stop,
                        )
                        if stop:
                            b = seg_list[seg_i][1]
                            nc.vector.tensor_add(
                                m_sb[:, b * c.H : (b + 1) * c.H],
                                pacc[:],
                                m_sb[:, b * c.H : (b + 1) * c.H],
                            )
                            seg_i += 1
                            seg_left = (
                                seg_list[seg_i][2] if seg_i < len(seg_list) else 0
                            )

            # ---------- helper: post-pass
            def postpass(l):
                bb = b1b if l == 0 else b2b
                dst = h1s if l == 0 else h2
                for t in range(c.NT):
                    sl = slice(t * c.H, (t + 1) * c.H)
                    t1 = mpool.tile([128, c.H], f32, tag="pp1")
                    nc.vector.tensor_scalar_mul(t1[:], m_sb[:, sl], nit[:, t : t + 1])
                    t2 = mpool.tile([128, c.H], f32, tag="pp2")
                    nc.vector.tensor_add(t2[:], t1[:], bb[:])
                    if l == 0:
                        nc.scalar.activation(
                            dst[:, sl], t2[:], AF.Relu, scale=not_[:, t : t + 1]
                        )
                    else:
                        nc.scalar.activation(dst[:, sl], t2[:], AF.Relu)

            # ================= layer 1 =================
            table_build(0)
            nc.gpsimd.collective_compute(
                "AllGather",
                mybir.AluOpType.bypass,
                replica_groups=[list(range(c.C))],
                ins=[bounce[0][:, :]],
                outs=[tbl[0][:, :]],
            )
            aggregate(0)
            if dbg:
                for b in range(c.NB):
                    mdt = mpool.tile([128, c.H], f32, tag="mdump")
                    nc.vector.tensor_copy(mdt[:], m_sb[:, b * c.H : (b + 1) * c.H])
                    nc.sync.dma_start(
                        out=mdump_out[b * 128 : (b + 1) * 128, :], in_=mdt[:]
                    )
            postpass(0)
            if dbg:
                for t in range(c.NT):
                    hdt = mpool.tile([128, c.H], f32, tag="hdump")
                    nc.vector.tensor_copy(hdt[:], h1s[:, t * c.H : (t + 1) * c.H])
                    nc.sync.dma_start(
                        out=hdump_out[t * 128 : (t + 1) * 128, :], in_=hdt[:]
                    )

            # ================= layer 2 =================
            table_build(1)
            nc.gpsimd.collective_compute(
                "AllGather",
                mybir.AluOpType.bypass,
                replica_groups=[list(range(c.C))],
                ins=[bounce[1][:, :]],
                outs=[tbl[1][:, :]],
            )
            aggregate(1)
            postpass(1)

            # ================= attention + pooling =================
            ppool = psacc.tile([c.H, c.G], f32, tag="ps_m")
            for t in range(c.NT):
                sl = slice(t * c.H, (t + 1) * c.H)
                tmp = mpool.tile([128, c.H], f32, tag="at1")
                nc.vector.tensor_mul(tmp[:], h2[:, sl], awb[:])
                red = mpool.tile([128, 1], f32, tag="at2")
                nc.vector.reduce_sum(red[:], tmp[:], axis=mybir.AxisListType.X)
                hwt = mpool.tile([128, 1], f32, tag="at3")
                nc.scalar.activation(hwt[:], red[:], AF.Sigmoid, bias=attnb[:])
                lo = t * 128
                nc.sync.dma_start(out=hw_out[lo : lo + 128, :], in_=hwt[:])
                hg = mpool.tile([128, c.H], f32, tag="at4")
                nc.vector.tensor_scalar_mul(hg[:], h2[:, sl], hwt[:])
                go = mpool.tile([128, c.G], f32, tag="at5")
                nc.sync.dma_start(out=go[:], in_=goh_in[lo : lo + 128, :])
                nc.tensor.matmul(
                    ppool[:], hg[:], go[:], start=(t == 0), stop=(t == c.NT - 1)
                )
            pooled = mpool.tile([c.H, c.G], f32, tag="pooled")
            nc.vector.tensor_copy(pooled[:], ppool[:])
            nc.sync.dma_start(out=ar_in[:, :], in_=pooled[:])
            nc.gpsimd.collective_compute(
                "AllReduce",
                mybir.AluOpType.add,
                replica_groups=[list(range(c.C))],
                ins=[ar_in[:, :]],
                outs=[ar_out[:, :]],
            )
            hgT = mpool.tile([c.H, c.G], f32, tag="hgT")
            nc.sync.dma_start(out=hgT[:], in_=ar_out[:, :])
            hgTs = mpool.tile([c.H, c.G], f32, tag="hgTs")
            nc.vector.tensor_mul(hgTs[:], hgT[:], icb[:])
            pout = psp.tile([c.G, c.NCLS], f32, tag="ps_tbl")
            nc.tensor.matmul(pout[:], hgTs[:], clsW[:], start=True, stop=True)
            ob = mpool.tile([c.G, c.NCLS], f32, tag="outb")
            nc.vector.tensor_add(ob[:], pout[:], clsbb[:])
            nc.sync.dma_start(out=out_out[:, :], in_=ob[:])

    nc.compile()
    return nc


# ----------------------------------------------------------------- driver ---
def _const_inputs(cfg, W1, b1, W2, b2, attn_W, attn_b, cls_W, cls_b, inv_cnt):
    import ml_dtypes

    c = cfg
    consts = {
        "W1": np.asarray(W1, np.float32),
        "W2": np.asarray(W2, np.float32),
        "b1b": np.tile(np.asarray(b1, np.float32)[None, :], (128, 1)),
        "b2b": np.tile(np.asarray(b2, np.float32)[None, :], (128, 1)),
        "awb": np.tile(np.asarray(attn_W, np.float32)[:, 0][None, :], (128, 1)),
        "clsW": np.asarray(cls_W, np.float32),
        "clsbb": np.tile(np.asarray(cls_b, np.float32)[None, :], (c.G, 1)),
        "icb": np.tile(inv_cnt[None, :], (c.H, 1)),
        "attnb": np.full((128, 1), np.float32(np.asarray(attn_b).reshape(-1)[0])),
        "ident": np.eye(128, dtype=np.float32),
        "id2": np.vstack(
            [np.eye(128, dtype=np.float32), np.zeros((1, 128), np.float32)]
        ).astype(ml_dtypes.bfloat16),
    }
    return consts


def run(x, edge_index, graph_id, W1, b1, W2, b2, attn_W, attn_b, cls_W, cls_b,
        cfg=None):
    from concourse.bass_utils import run_bass_kernel_spmd

    cfg = cfg or Cfg()
    in_maps, meta = preprocess(x, edge_index, graph_id, cfg)
    consts = _const_inputs(
        cfg, W1, b1, W2, b2, attn_W, attn_b, cls_W, cls_b, meta["inv_cnt"]
    )
    for m in in_maps:
        m.update(consts)

    ckey = (cfg.N, cfg.E, tuple(map(tuple, meta["T"])), len(meta["calls"]))
    if ckey not in _KCACHE:
        _KCACHE[ckey] = build_kernel(cfg, meta["T"], meta["calls"], meta["CK"])
    nc = _KCACHE[ckey]

    res = run_bass_kernel_spmd(
        nc, in_maps, core_ids=list(range(cfg.C)),
        trace=os.environ.get("KTRACE", "0") == "1",
    )
    out = res.results[0]["out"]
    hw = np.concatenate(
        [res.results[cc]["hw"][: cfg.NP] for cc in range(cfg.C)], axis=0
    )
    return out, hw, res


def kernel(**inputs):
    out, hw, _ = run(**inputs)
    return out, hw
